# revision 22
# baseline (speedup 1.0000x reference)
"""Trainium2 Bass kernel for nn_DenseGeometryEdgeHead.

Math background
---------------
reference(q, coords, ...) computes, per batch b:

    logits[i,j] = (q W) q^T  +  g(dx_ij, dy_ij)  + bias,   then diag-zero + mask

where g() is a tiny MLP (6->64->64->1) applied to pairwise geometry features
that depend ONLY on (dx, dy) = p_i - p_j.

Because g is a function of a coordinate *difference*, it factors through
Fourier features:  g(u,v) ~= sum_m 2*Re[c_m e^{i(w_m . p_i)} e^{-i(w_m . p_j)}].
Each mode contributes a rank-2 term cos/sin(theta_i)*cos/sin(theta_j), so the
whole geo-MLP collapses into a low-rank bilinear form  Phi(p_i) . Psi(p_j).
Stacked with the q W q^T term this makes the ENTIRE kernel one matmul with
contraction K = 256 (bilinear) + 1 (constants) + 2*NMODE (fourier) <= 768.

The Fourier series is fit on the host per call (it depends only on the tiny
MLP weights): sample g on a grid, mollify the angular near-origin structure
below radius R0, taper for periodicity, FFT, keep the strongest NMODE modes.
Pairs with r < R0 (a ~2% sliver) get exact host-computed corrections that are
scatter-added into the output, so the approximation error lives only in the
smooth far-field where the truncated series is accurate.

Device work per core (data-parallel over (batch, row-half)):
    out[512,1024] = AT^T @ BT   (K=768, fp32 PSUM)  + active-mask -1e9 select
which is memory-bound (~7 MB of HBM traffic per core).
"""

import numpy as np

import concourse.bass as bass
import concourse.bacc as bacc
import concourse.mybir as mybir
import concourse.tile as tile
from concourse.bass_utils import run_bass_kernel_spmd

# ---- problem constants (hardcoded; harness runs kernel.py standalone) ----
B, Q, D, H = 4, 1024, 256, 64
NCORES = 8
NSLICE = 6               # K slices of 128  -> total contraction 768
KTOT = NSLICE * 128      # 768 = 256 (qW/q) + 1 (const) + 510 (255 modes) + 1 pad
NMODE = 255
RHALF = Q // 2           # output rows per core

# ---- fourier fit hyperparameters (host-side, tunable without recompiling) ----
FIT_NG = 512             # fit grid resolution
FIT_L = 1.28             # series half-period (u,v in [-1,1])
FIT_KMAX = 24            # candidate mode box
FIT_R0 = 0.18            # below this radius: mollified + exact corrections
FIT_RIN_FRAC = 0.5       # blend starts at R0*frac
FIT_TAPER_LO = 1.02      # periodicity taper window
FIT_TAPER_HI = 1.26
FIT_NSUB = 60000         # least-squares refit sample count
FIT_IRLS = 3             # IRLS iterations toward minimax

F32 = mybir.dt.float32
F32R = mybir.dt.float32r  # full-rate PE streaming (1 cyc/row vs 4 for fp32)

# True: q/qW contraction slices run in native fp32 (2-pass, slower, exact);
# False: everything float32r (~12-bit multiply mantissa, max abs err ~0.016)
PRECISE_BILINEAR = False

_LAST_RESULT = None      # BassKernelResults of the most recent device run
_LAST_FIT = None         # diagnostics from the most recent fourier fit
_NC_CACHE = None


# ======================= device kernel =======================

def _build_device():
    """One core's program: out[512,1024] = AT^T @ BT with mask/-1e9 select."""
    global _NC_CACHE
    if _NC_CACHE is not None:
        return _NC_CACHE
    nc = bacc.Bacc()  # Bacc.finalize() legalizes multi-wait instructions
    # combined per-K-slice chunks: [ks][p, 0:512]=AT slice, [p, 512:1536]=BT
    # slice — so each matmul depends on exactly ONE DMA (PE instructions
    # only support a single sync-wait).
    ab_d = nc.declare_dram_parameter("ab", [NSLICE, 128, RHALF + Q], F32R, False)
    # mask pack: cols [0:Q]=amj bcast, [Q]=-1e9, [Q+1:Q+5]=ami, [Q+5:Q+9]=amiadd
    am_d = nc.declare_dram_parameter("am", [128, Q + 9], F32, False)
    out_d = nc.declare_dram_parameter("out", [RHALF, Q], F32, True)

    Id = mybir.ActivationFunctionType.Identity
    mult = mybir.AluOpType.mult
    add = mybir.AluOpType.add

    with tile.TileContext(nc) as tc:
        with (
            tc.tile_pool(name="inp", bufs=1) as inp,
            tc.tile_pool(name="psum", bufs=3, space=bass.MemorySpace.PSUM) as psum,
            tc.tile_pool(name="work", bufs=3) as work,
        ):
            ats, bts = [], []
            for ks in range(NSLICE):
                dt_k = F32 if (PRECISE_BILINEAR and ks < 2) else F32R
                ab = inp.tile([128, RHALF + Q], dt_k, tag=f"ab{ks}")
                # split each chunk across BOTH HWDGE queues (sync + scalar)
                # so the load stream isn't serialized on one ring
                half = (RHALF + Q) // 2
                nc.sync.dma_start(ab[:, :half], ab_d[ks, :, :half])
                nc.scalar.dma_start(ab[:, half:], ab_d[ks, :, half:])
                ats.append(ab[:, :RHALF])
                bts.append(ab[:, RHALF:])
            am = inp.tile([128, Q + 9], F32, tag="am")
            nc.scalar.dma_start(am[:], am_d[:, :])
            amj = am[:, :Q]
            neg = am[:, Q : Q + 1]

            # pre-observe the mask DMA on the vector engine so later DVE ops
            # only carry the PE-sem wait (1-wait limit per instruction)
            warm = work.tile([128, 1], F32, tag="warm")
            nc.vector.tensor_copy(warm[:], neg[:])

            # amja = (amj - 1) * 1e9   (exact: {0,-1e9})
            amja = inp.tile([128, Q], F32, tag="amja")
            nc.scalar.activation(amja[:], amj[:], Id, bias=neg[:], scale=1e9)

            import bass_rust as _br
            prev_last_mm = None
            for ib in range(4):
                isl = slice(ib * 128, (ib + 1) * 128)
                ami_c = am[:, Q + 1 + ib : Q + 2 + ib]
                amia_c = am[:, Q + 5 + ib : Q + 6 + ib]
                # both j-halves accumulate into one 2-bank psum tile
                ps = psum.tile([128, Q], F32, tag="ps")
                first_mm = last_mm = None
                for ks in range(NSLICE):
                    for jc in range(2):
                        jsl = slice(jc * RHALF, (jc + 1) * RHALF)
                        mm = nc.tensor.matmul(
                            ps[:, jsl],
                            ats[ks][:, isl],
                            bts[ks][:, jsl],
                            start=(ks == 0),
                            stop=(ks == NSLICE - 1),
                        )
                        last_mm = mm
                        if first_mm is None:
                            first_mm = mm
                # keep PE group-contiguous: without this the scheduler
                # round-robins the psum groups and no group's mask chain can
                # start until nearly the end
                if prev_last_mm is not None:
                    _br.add_dep_helper(
                        first_mm.ins, prev_last_mm.ins, True,
                        "serialize psum groups for early drain",
                    )
                prev_last_mm = last_mm
                # x = psum * ami_row * amj   ({logits or 0})
                x = work.tile([128, Q], F32, tag="x")
                nc.vector.scalar_tensor_tensor(
                    x[:], ps[:], ami_c, amj[:], op0=mult, op1=mult,
                )
                # y = amja * ami_row + x     ({logits, 0, -1e9})
                y = work.tile([128, Q], F32, tag="y")
                nc.vector.scalar_tensor_tensor(
                    y[:], amja[:], ami_c, x[:], op0=mult, op1=add,
                )
                # o = y + (ami_row - 1)*1e9  (rows with ami=0 -> -1e9)
                o = work.tile([128, Q], F32, tag="o")
                nc.scalar.activation(o[:], y[:], Id, bias=amia_c)
                nc.sync.dma_start(out_d[isl, :], o[:])
    nc.finalize()
    _NC_CACHE = nc
    return nc


# ======================= host-side math =======================

def _silu(x):
    return x * (1.0 / (1.0 + np.exp(-x)))


def _mlp(feats, w1, b1, w2, b2, w3, b3):
    """feats [..., 6] -> geo logits [...] (includes +b3)."""
    h = _silu(feats @ w1 + b1)
    h = _silu(h @ w2 + b2)
    return (h @ w3)[..., 0] + b3[0]


def _g_exact(u, v, w1, b1, w2, b2, w3, b3):
    """Exact reference-semantics geo MLP output for difference vectors."""
    r2 = u * u + v * v
    rd = np.sqrt(r2 + 1e-8)
    mz = (np.abs(u) < 1e-6) & (np.abs(v) < 1e-6)
    us = np.where(mz, 1e-6, u)
    vs = np.where(mz, 1e-6, v)
    ang = np.arctan2(vs, us)
    feats = np.stack([u, v, rd, r2, np.sin(ang), np.cos(ang)], axis=-1)
    return _mlp(feats, w1, b1, w2, b2, w3, b3)


def _qstep(t):
    t = np.clip(t, 0.0, 1.0)
    return t * t * t * (t * (6.0 * t - 15.0) + 10.0)


def _fit_fourier(w1, b1, w2, b2, w3, b3):
    """Fit truncated 2-D fourier series to g on [-L,L)^2.

    Returns (dc, mus, mvs, coeffs, diag) — dc real, modes in the half-plane
    with complex coeffs (term = 2*Re[c * e^{i pi (mu u + mv v)/L}]).
    """
    NG, L = FIT_NG, FIT_L
    du = 2.0 * L / NG
    ax = (np.arange(NG) - NG // 2) * du
    U = np.broadcast_to(ax[None, :], (NG, NG))      # u along columns
    V = np.broadcast_to(ax[:, None], (NG, NG))      # v along rows
    r = np.hypot(U, V)
    r2 = U * U + V * V
    rd = np.sqrt(r2 + 1e-8)

    # mollified angular features near origin
    blend = _qstep((r - FIT_R0 * FIT_RIN_FRAC) / (FIT_R0 * (1.0 - FIT_RIN_FRAC)))
    rsafe = np.maximum(r, 1e-30)
    sb = np.float64(np.sqrt(2.0) / 2.0)
    s_m = blend * (V / rsafe) + (1.0 - blend) * sb
    c_m = blend * (U / rsafe) + (1.0 - blend) * sb

    feats = np.stack([U, V, rd, r2, s_m, c_m], axis=-1).astype(np.float32)
    g_m = _mlp(feats.reshape(-1, 6), w1.astype(np.float32), b1.astype(np.float32),
               w2.astype(np.float32), b2.astype(np.float32),
               w3.astype(np.float32), b3.astype(np.float32)).reshape(NG, NG)
    g_m = g_m.astype(np.float64)

    # periodicity taper
    wu = 1.0 - _qstep((np.abs(ax) - FIT_TAPER_LO) / (FIT_TAPER_HI - FIT_TAPER_LO))
    Wt = wu[:, None] * wu[None, :]
    ring = (Wt > 0.05) & (Wt < 0.95)
    C = float(np.mean(g_m[ring])) if ring.any() else float(np.mean(g_m))
    gp = Wt * (g_m - C) + C

    F = np.fft.fft2(gp) / (NG * NG)
    fr = np.fft.fftfreq(NG, d=1.0 / NG).astype(np.int64)  # signed integer freqs
    # c_{mv,mu} = F[m,n] * (-1)^(m~+n~);  series = sum c e^{i pi (mu u + mv v)/L}
    sgn = np.where(fr % 2 == 0, 1.0, -1.0)
    Csym = F * sgn[:, None] * sgn[None, :]

    # candidate half-plane modes within the KMAX box
    KM = FIT_KMAX
    cand = []
    for mi, mv in enumerate(fr):
        if not (0 <= mv <= KM):
            continue
        for ni, mu in enumerate(fr):
            if abs(mu) > KM:
                continue
            if mv == 0 and mu <= 0:
                continue
            cand.append((abs(Csym[mi, ni]), mu, mv, Csym[mi, ni]))
    cand.sort(key=lambda t: -t[0])
    sel = cand[:NMODE]
    mus = np.array([t[1] for t in sel], np.int64)
    mvs = np.array([t[2] for t in sel], np.int64)
    dc = float(Csym[0, 0].real)

    # ---- least-squares refit of (dc, c_m) on the region that matters:
    # {r >= R0, |u|,|v| <= 1}; FFT coeffs optimize the wrong (periodic) norm.
    far = (r >= FIT_R0) & (np.abs(U) <= 1.0) & (np.abs(V) <= 1.0)
    g_true = _g_exact(U[far], V[far], w1, b1, w2, b2, w3, b3)
    rng = np.random.default_rng(12345)
    nfar = g_true.size
    sub = rng.choice(nfar, size=min(FIT_NSUB, nfar), replace=False)
    uu, vv, gg = U[far][sub], V[far][sub], g_true[sub]
    w = np.pi / L
    PH = np.exp(1j * w * (np.outer(uu, mus) + np.outer(vv, mvs)))
    # 2*Re[c e^{i th}] = 2*Re(c)*cos - 2*Im(c)*sin
    X = np.concatenate([np.ones((uu.size, 1)), 2 * PH.real, -2 * PH.imag], axis=1)
    wt = np.ones(uu.size)
    sol = None
    for _ in range(1 + FIT_IRLS):
        Xw = X * wt[:, None]
        G = Xw.T @ X
        sol = np.linalg.solve(G + 1e-9 * np.eye(G.shape[0]), Xw.T @ gg)
        err = X @ sol - gg
        wt = (np.abs(err) + 1e-4)
    dc = float(sol[0])
    cs = sol[1 : 1 + NMODE] + 1j * sol[1 + NMODE :]
    err = X @ sol - gg
    diag = dict(max_err=float(np.abs(err).max()),
                p99_err=float(np.quantile(np.abs(err), 0.99)))
    return dc, mus, mvs, cs, diag


def _series_eval(u, v, dc, mus, mvs, cs):
    """Evaluate the selected truncated series at difference vectors (f64)."""
    out = np.full(u.shape, dc, np.float64)
    w = np.pi / FIT_L
    step = 1 << 14
    for s in range(0, u.size, step):
        sl = slice(s, min(s + step, u.size))
        ph = np.exp(1j * w * (np.outer(u.ravel()[sl], mus) + np.outer(v.ravel()[sl], mvs)))
        out.ravel()[sl] += 2.0 * np.real(ph @ cs)
    return out


# ======================= kernel entry =======================

def kernel(q, coords, active_mask, W, w1, b1, w2, b2, w3, b3, bias):
    global _LAST_RESULT, _LAST_FIT
    q = np.asarray(q, np.float32)
    coords = np.asarray(coords, np.float32)
    am = np.asarray(active_mask).astype(bool)
    W = np.asarray(W, np.float64)
    w1, b1 = np.asarray(w1, np.float64), np.asarray(b1, np.float64)
    w2, b2 = np.asarray(w2, np.float64), np.asarray(b2, np.float64)
    w3, b3 = np.asarray(w3, np.float64), np.asarray(b3, np.float64)
    bias0 = float(np.asarray(bias).reshape(-1)[0])

    # ---- fourier factorization of the geo MLP ----
    dc, mus, mvs, cs, fit_diag = _fit_fourier(w1, b1, w2, b2, w3, b3)
    _LAST_FIT = fit_diag

    # ---- per-point feature maps ----
    # AT rows: [qW^T (256) | const row | 2rho cos(th+g), 2rho sin(th+g) | pad]
    # BT rows: [q^T  (256) | ones      | cos(th), sin(th)              | pad]
    qW = (q.astype(np.float64) @ W).astype(np.float64)          # [B,Q,D]
    x = coords[..., 0].astype(np.float64)                        # [B,Q]
    y = coords[..., 1].astype(np.float64)
    wfreq = np.pi / FIT_L
    theta = wfreq * (x[..., None] * mus + y[..., None] * mvs)    # [B,Q,NMODE]
    rho = np.abs(cs)
    gam = np.angle(cs)
    AT = np.zeros((B, KTOT, Q), np.float32)
    BT = np.zeros((B, KTOT, Q), np.float32)
    for b in range(B):
        AT[b, :D, :] = qW[b].T.astype(np.float32)
        BT[b, :D, :] = q[b].T
        AT[b, D, :] = np.float32(dc + b3[0] + bias0)
        BT[b, D, :] = 1.0
        AT[b, D + 1 : D + 1 + NMODE, :] = (2.0 * rho[:, None] * np.cos(theta[b].T + gam[:, None])).astype(np.float32)
        AT[b, D + 1 + NMODE : D + 1 + 2 * NMODE, :] = (2.0 * rho[:, None] * np.sin(theta[b].T + gam[:, None])).astype(np.float32)
        BT[b, D + 1 : D + 1 + NMODE, :] = np.cos(theta[b].T).astype(np.float32)
        BT[b, D + 1 + NMODE : D + 1 + 2 * NMODE, :] = np.sin(theta[b].T).astype(np.float32)

    # ---- per-core inputs: core c -> (batch c//2, row half c%2) ----
    amf = am.astype(np.float32)
    in_maps = []
    for c in range(NCORES):
        b, ih = c // 2, c % 2
        rs = slice(ih * RHALF, (ih + 1) * RHALF)
        ami = amf[b, rs].reshape(4, 128).T                       # [128,4]
        ab_c = np.concatenate(
            [AT[b, :, rs].reshape(NSLICE, 128, RHALF), BT[b].reshape(NSLICE, 128, Q)],
            axis=2,
        )
        am_c = np.empty((128, Q + 9), np.float32)
        am_c[:, :Q] = amf[b]
        am_c[:, Q] = -1e9
        am_c[:, Q + 1 : Q + 5] = ami
        am_c[:, Q + 5 : Q + 9] = (ami - 1.0) * 1e9
        in_maps.append({
            "ab": np.ascontiguousarray(ab_c),
            "am": am_c,
        })

    nc = _build_device()
    res = run_bass_kernel_spmd(nc, in_maps, core_ids=list(range(NCORES)))
    _LAST_RESULT = res

    full = np.empty((B, Q, Q), np.float32)
    for c in range(NCORES):
        b, ih = c // 2, c % 2
        full[b, ih * RHALF : (ih + 1) * RHALF, :] = res.results[c]["out"]

    # ---- exact corrections for near-origin pairs (r < R0) ----
    ar = np.arange(Q)
    for b in range(B):
        u = x[b][:, None] - x[b][None, :]
        v = y[b][:, None] - y[b][None, :]
        mm = am[b][:, None] & am[b][None, :]
        near = (u * u + v * v < FIT_R0 * FIT_R0) & mm
        near[ar, ar] = False
        ii, jj = np.nonzero(near)
        if ii.size:
            uu, vv = u[ii, jj], v[ii, jj]
            corr = _g_exact(uu, vv, w1, b1, w2, b2, w3, b3) \
                 - _series_eval(uu, vv, dc, mus, mvs, cs)
            full[b, ii, jj] += corr.astype(np.float32)
        # diagonal: 0 where active, -1e9 where masked
        full[b, ar, ar] = np.where(am[b], np.float32(0.0), np.float32(-1e9))

    return full


# revision 24
# speedup vs baseline: 1.1653x; 1.1653x over previous
"""Trainium2 Bass kernel for nn_DenseGeometryEdgeHead.

Math background
---------------
reference(q, coords, ...) computes, per batch b:

    logits[i,j] = (q W) q^T  +  g(dx_ij, dy_ij)  + bias,   then diag-zero + mask

where g() is a tiny MLP (6->64->64->1) applied to pairwise geometry features
that depend ONLY on (dx, dy) = p_i - p_j.

Because g is a function of a coordinate *difference*, it factors through
Fourier features:  g(u,v) ~= sum_m 2*Re[c_m e^{i(w_m . p_i)} e^{-i(w_m . p_j)}].
Each mode contributes a rank-2 term cos/sin(theta_i)*cos/sin(theta_j), so the
whole geo-MLP collapses into a low-rank bilinear form  Phi(p_i) . Psi(p_j).
Stacked with the q W q^T term this makes the ENTIRE kernel one matmul with
contraction K = 256 (bilinear) + 1 (constants) + 2*NMODE (fourier) <= 768.

The Fourier series is fit on the host per call (it depends only on the tiny
MLP weights): sample g on a grid, mollify the angular near-origin structure
below radius R0, taper for periodicity, FFT, keep the strongest NMODE modes.
Pairs with r < R0 (a ~2% sliver) get exact host-computed corrections that are
scatter-added into the output, so the approximation error lives only in the
smooth far-field where the truncated series is accurate.

Device work per core (data-parallel over (batch, row-half)):
    out[512,1024] = AT^T @ BT   (K=768, fp32 PSUM)  + active-mask -1e9 select
which is memory-bound (~7 MB of HBM traffic per core).
"""

import numpy as np

import concourse.bass as bass
import concourse.bacc as bacc
import concourse.mybir as mybir
import concourse.tile as tile
from concourse.bass_utils import run_bass_kernel_spmd

# ---- problem constants (hardcoded; harness runs kernel.py standalone) ----
B, Q, D, H = 4, 1024, 256, 64
NCORES = 8
NSLICE = 4               # K slices of 128  -> total contraction 512
KTOT = NSLICE * 128      # 512 = 256 (qW/q) + 1 (const) + 254 (127 modes) + 1 pad
NMODE = 127
RHALF = Q // 2           # output rows per core

# ---- fourier fit hyperparameters (host-side, tunable without recompiling) ----
FIT_NG = 512             # fit grid resolution
FIT_L = 1.28             # series half-period (u,v in [-1,1])
FIT_KMAX = 24            # candidate mode box
FIT_R0 = 0.22            # below this radius: mollified + exact corrections
FIT_RIN_FRAC = 0.5       # blend starts at R0*frac
FIT_TAPER_LO = 1.02      # periodicity taper window
FIT_TAPER_HI = 1.26
FIT_NSUB = 60000         # least-squares refit sample count
FIT_IRLS = 3             # IRLS iterations toward minimax

F32 = mybir.dt.float32
F32R = mybir.dt.float32r  # full-rate PE streaming (1 cyc/row vs 4 for fp32)

# True: q/qW contraction slices run in native fp32 (2-pass, slower, exact);
# False: everything float32r (~12-bit multiply mantissa, max abs err ~0.016)
PRECISE_BILINEAR = False

_LAST_RESULT = None      # BassKernelResults of the most recent device run
_LAST_FIT = None         # diagnostics from the most recent fourier fit
_NC_CACHE = None


# ======================= device kernel =======================

def _build_device():
    """One core's program: out[512,1024] = AT^T @ BT with mask/-1e9 select."""
    global _NC_CACHE
    if _NC_CACHE is not None:
        return _NC_CACHE
    nc = bacc.Bacc()  # Bacc.finalize() legalizes multi-wait instructions
    # combined per-K-slice chunks: [ks][p, 0:512]=AT slice, [p, 512:1536]=BT
    # slice — so each matmul depends on exactly ONE DMA (PE instructions
    # only support a single sync-wait).
    ab_d = nc.declare_dram_parameter("ab", [NSLICE, 128, RHALF + Q], F32R, False)
    # mask pack: cols [0:Q]=amj bcast, [Q]=-1e9, [Q+1:Q+5]=ami, [Q+5:Q+9]=amiadd
    am_d = nc.declare_dram_parameter("am", [128, Q + 9], F32, False)
    out_d = nc.declare_dram_parameter("out", [RHALF, Q], F32, True)

    Id = mybir.ActivationFunctionType.Identity
    mult = mybir.AluOpType.mult
    add = mybir.AluOpType.add

    with tile.TileContext(nc) as tc:
        with (
            tc.tile_pool(name="inp", bufs=1) as inp,
            tc.tile_pool(name="psum", bufs=3, space=bass.MemorySpace.PSUM) as psum,
            tc.tile_pool(name="work", bufs=3) as work,
        ):
            ats, bts = [], []
            for ks in range(NSLICE):
                dt_k = F32 if (PRECISE_BILINEAR and ks < 2) else F32R
                ab = inp.tile([128, RHALF + Q], dt_k, tag=f"ab{ks}")
                # split each chunk across BOTH HWDGE queues (sync + scalar)
                # so the load stream isn't serialized on one ring
                half = (RHALF + Q) // 2
                nc.sync.dma_start(ab[:, :half], ab_d[ks, :, :half])
                nc.scalar.dma_start(ab[:, half:], ab_d[ks, :, half:])
                ats.append(ab[:, :RHALF])
                bts.append(ab[:, RHALF:])
            am = inp.tile([128, Q + 9], F32, tag="am")
            nc.scalar.dma_start(am[:], am_d[:, :])
            amj = am[:, :Q]
            neg = am[:, Q : Q + 1]

            # pre-observe the mask DMA on the vector engine so later DVE ops
            # only carry the PE-sem wait (1-wait limit per instruction)
            warm = work.tile([128, 1], F32, tag="warm")
            nc.vector.tensor_copy(warm[:], neg[:])

            # amja = (amj - 1) * 1e9   (exact: {0,-1e9})
            amja = inp.tile([128, Q], F32, tag="amja")
            nc.scalar.activation(amja[:], amj[:], Id, bias=neg[:], scale=1e9)

            import bass_rust as _br
            prev_last_mm = None
            for ib in range(4):
                isl = slice(ib * 128, (ib + 1) * 128)
                ami_c = am[:, Q + 1 + ib : Q + 2 + ib]
                amia_c = am[:, Q + 5 + ib : Q + 6 + ib]
                # both j-halves accumulate into one 2-bank psum tile
                ps = psum.tile([128, Q], F32, tag="ps")
                first_mm = last_mm = None
                for ks in range(NSLICE):
                    for jc in range(2):
                        jsl = slice(jc * RHALF, (jc + 1) * RHALF)
                        mm = nc.tensor.matmul(
                            ps[:, jsl],
                            ats[ks][:, isl],
                            bts[ks][:, jsl],
                            start=(ks == 0),
                            stop=(ks == NSLICE - 1),
                        )
                        last_mm = mm
                        if first_mm is None:
                            first_mm = mm
                # keep PE group-contiguous: without this the scheduler
                # round-robins the psum groups and no group's mask chain can
                # start until nearly the end
                if prev_last_mm is not None:
                    _br.add_dep_helper(
                        first_mm.ins, prev_last_mm.ins, True,
                        "serialize psum groups for early drain",
                    )
                prev_last_mm = last_mm
                # row mask on ACT straight out of PSUM:
                # o1 = ps*ami + (ami-1)*1e9   ({logits or -1e9 rows})
                o1 = work.tile([128, Q], F32, tag="o1")
                nc.scalar.activation(o1[:], ps[:], Id, scale=ami_c, bias=amia_c)
                # col mask on DVE: out = o1*amj + (amj-1)*1e9
                t = work.tile([128, Q], F32, tag="t")
                nc.vector.tensor_mul(t[:], o1[:], amj[:])
                o = work.tile([128, Q], F32, tag="o")
                nc.vector.tensor_add(o[:], t[:], amja[:])
                nc.sync.dma_start(out_d[isl, :], o[:])
    nc.finalize()
    _NC_CACHE = nc
    return nc


# ======================= host-side math =======================

def _silu(x):
    return x * (1.0 / (1.0 + np.exp(-x)))


def _mlp(feats, w1, b1, w2, b2, w3, b3):
    """feats [..., 6] -> geo logits [...] (includes +b3)."""
    h = _silu(feats @ w1 + b1)
    h = _silu(h @ w2 + b2)
    return (h @ w3)[..., 0] + b3[0]


def _g_exact(u, v, w1, b1, w2, b2, w3, b3):
    """Exact reference-semantics geo MLP output for difference vectors."""
    r2 = u * u + v * v
    rd = np.sqrt(r2 + 1e-8)
    mz = (np.abs(u) < 1e-6) & (np.abs(v) < 1e-6)
    us = np.where(mz, 1e-6, u)
    vs = np.where(mz, 1e-6, v)
    ang = np.arctan2(vs, us)
    feats = np.stack([u, v, rd, r2, np.sin(ang), np.cos(ang)], axis=-1)
    return _mlp(feats, w1, b1, w2, b2, w3, b3)


def _qstep(t):
    t = np.clip(t, 0.0, 1.0)
    return t * t * t * (t * (6.0 * t - 15.0) + 10.0)


def _fit_fourier(w1, b1, w2, b2, w3, b3):
    """Fit truncated 2-D fourier series to g on [-L,L)^2.

    Returns (dc, mus, mvs, coeffs, diag) — dc real, modes in the half-plane
    with complex coeffs (term = 2*Re[c * e^{i pi (mu u + mv v)/L}]).
    """
    NG, L = FIT_NG, FIT_L
    du = 2.0 * L / NG
    ax = (np.arange(NG) - NG // 2) * du
    U = np.broadcast_to(ax[None, :], (NG, NG))      # u along columns
    V = np.broadcast_to(ax[:, None], (NG, NG))      # v along rows
    r = np.hypot(U, V)
    r2 = U * U + V * V
    rd = np.sqrt(r2 + 1e-8)

    # mollified angular features near origin
    blend = _qstep((r - FIT_R0 * FIT_RIN_FRAC) / (FIT_R0 * (1.0 - FIT_RIN_FRAC)))
    rsafe = np.maximum(r, 1e-30)
    sb = np.float64(np.sqrt(2.0) / 2.0)
    s_m = blend * (V / rsafe) + (1.0 - blend) * sb
    c_m = blend * (U / rsafe) + (1.0 - blend) * sb

    feats = np.stack([U, V, rd, r2, s_m, c_m], axis=-1).astype(np.float32)
    g_m = _mlp(feats.reshape(-1, 6), w1.astype(np.float32), b1.astype(np.float32),
               w2.astype(np.float32), b2.astype(np.float32),
               w3.astype(np.float32), b3.astype(np.float32)).reshape(NG, NG)
    g_m = g_m.astype(np.float64)

    # periodicity taper
    wu = 1.0 - _qstep((np.abs(ax) - FIT_TAPER_LO) / (FIT_TAPER_HI - FIT_TAPER_LO))
    Wt = wu[:, None] * wu[None, :]
    ring = (Wt > 0.05) & (Wt < 0.95)
    C = float(np.mean(g_m[ring])) if ring.any() else float(np.mean(g_m))
    gp = Wt * (g_m - C) + C

    F = np.fft.fft2(gp) / (NG * NG)
    fr = np.fft.fftfreq(NG, d=1.0 / NG).astype(np.int64)  # signed integer freqs
    # c_{mv,mu} = F[m,n] * (-1)^(m~+n~);  series = sum c e^{i pi (mu u + mv v)/L}
    sgn = np.where(fr % 2 == 0, 1.0, -1.0)
    Csym = F * sgn[:, None] * sgn[None, :]

    # candidate half-plane modes within the KMAX box
    KM = FIT_KMAX
    cand = []
    for mi, mv in enumerate(fr):
        if not (0 <= mv <= KM):
            continue
        for ni, mu in enumerate(fr):
            if abs(mu) > KM:
                continue
            if mv == 0 and mu <= 0:
                continue
            cand.append((abs(Csym[mi, ni]), mu, mv, Csym[mi, ni]))
    cand.sort(key=lambda t: -t[0])
    sel = cand[:NMODE]
    mus = np.array([t[1] for t in sel], np.int64)
    mvs = np.array([t[2] for t in sel], np.int64)
    dc = float(Csym[0, 0].real)

    # ---- least-squares refit of (dc, c_m) on the region that matters:
    # {r >= R0, |u|,|v| <= 1}; FFT coeffs optimize the wrong (periodic) norm.
    far = (r >= FIT_R0) & (np.abs(U) <= 1.0) & (np.abs(V) <= 1.0)
    g_true = _g_exact(U[far], V[far], w1, b1, w2, b2, w3, b3)
    rng = np.random.default_rng(12345)
    nfar = g_true.size
    sub = rng.choice(nfar, size=min(FIT_NSUB, nfar), replace=False)
    uu, vv, gg = U[far][sub], V[far][sub], g_true[sub]
    w = np.pi / L
    PH = np.exp(1j * w * (np.outer(uu, mus) + np.outer(vv, mvs)))
    # 2*Re[c e^{i th}] = 2*Re(c)*cos - 2*Im(c)*sin
    X = np.concatenate([np.ones((uu.size, 1)), 2 * PH.real, -2 * PH.imag], axis=1)
    wt = np.ones(uu.size)
    sol = None
    for _ in range(1 + FIT_IRLS):
        Xw = X * wt[:, None]
        G = Xw.T @ X
        sol = np.linalg.solve(G + 1e-9 * np.eye(G.shape[0]), Xw.T @ gg)
        err = X @ sol - gg
        wt = (np.abs(err) + 1e-4)
    dc = float(sol[0])
    cs = sol[1 : 1 + NMODE] + 1j * sol[1 + NMODE :]
    err = X @ sol - gg
    diag = dict(max_err=float(np.abs(err).max()),
                p99_err=float(np.quantile(np.abs(err), 0.99)))
    return dc, mus, mvs, cs, diag


def _series_eval(u, v, dc, mus, mvs, cs):
    """Evaluate the selected truncated series at difference vectors (f64)."""
    out = np.full(u.shape, dc, np.float64)
    w = np.pi / FIT_L
    step = 1 << 14
    for s in range(0, u.size, step):
        sl = slice(s, min(s + step, u.size))
        ph = np.exp(1j * w * (np.outer(u.ravel()[sl], mus) + np.outer(v.ravel()[sl], mvs)))
        out.ravel()[sl] += 2.0 * np.real(ph @ cs)
    return out


# ======================= kernel entry =======================

def kernel(q, coords, active_mask, W, w1, b1, w2, b2, w3, b3, bias):
    global _LAST_RESULT, _LAST_FIT
    q = np.asarray(q, np.float32)
    coords = np.asarray(coords, np.float32)
    am = np.asarray(active_mask).astype(bool)
    W = np.asarray(W, np.float64)
    w1, b1 = np.asarray(w1, np.float64), np.asarray(b1, np.float64)
    w2, b2 = np.asarray(w2, np.float64), np.asarray(b2, np.float64)
    w3, b3 = np.asarray(w3, np.float64), np.asarray(b3, np.float64)
    bias0 = float(np.asarray(bias).reshape(-1)[0])

    # ---- fourier factorization of the geo MLP ----
    dc, mus, mvs, cs, fit_diag = _fit_fourier(w1, b1, w2, b2, w3, b3)
    _LAST_FIT = fit_diag

    # ---- per-point feature maps ----
    # AT rows: [qW^T (256) | const row | 2rho cos(th+g), 2rho sin(th+g) | pad]
    # BT rows: [q^T  (256) | ones      | cos(th), sin(th)              | pad]
    qW = (q.astype(np.float64) @ W).astype(np.float64)          # [B,Q,D]
    x = coords[..., 0].astype(np.float64)                        # [B,Q]
    y = coords[..., 1].astype(np.float64)
    wfreq = np.pi / FIT_L
    theta = wfreq * (x[..., None] * mus + y[..., None] * mvs)    # [B,Q,NMODE]
    rho = np.abs(cs)
    gam = np.angle(cs)
    AT = np.zeros((B, KTOT, Q), np.float32)
    BT = np.zeros((B, KTOT, Q), np.float32)
    for b in range(B):
        AT[b, :D, :] = qW[b].T.astype(np.float32)
        BT[b, :D, :] = q[b].T
        AT[b, D, :] = np.float32(dc + b3[0] + bias0)
        BT[b, D, :] = 1.0
        AT[b, D + 1 : D + 1 + NMODE, :] = (2.0 * rho[:, None] * np.cos(theta[b].T + gam[:, None])).astype(np.float32)
        AT[b, D + 1 + NMODE : D + 1 + 2 * NMODE, :] = (2.0 * rho[:, None] * np.sin(theta[b].T + gam[:, None])).astype(np.float32)
        BT[b, D + 1 : D + 1 + NMODE, :] = np.cos(theta[b].T).astype(np.float32)
        BT[b, D + 1 + NMODE : D + 1 + 2 * NMODE, :] = np.sin(theta[b].T).astype(np.float32)

    # ---- per-core inputs: core c -> (batch c//2, row half c%2) ----
    amf = am.astype(np.float32)
    in_maps = []
    for c in range(NCORES):
        b, ih = c // 2, c % 2
        rs = slice(ih * RHALF, (ih + 1) * RHALF)
        ami = amf[b, rs].reshape(4, 128).T                       # [128,4]
        ab_c = np.concatenate(
            [AT[b, :, rs].reshape(NSLICE, 128, RHALF), BT[b].reshape(NSLICE, 128, Q)],
            axis=2,
        )
        am_c = np.empty((128, Q + 9), np.float32)
        am_c[:, :Q] = amf[b]
        am_c[:, Q] = -1e9
        am_c[:, Q + 1 : Q + 5] = ami
        am_c[:, Q + 5 : Q + 9] = (ami - 1.0) * 1e9
        in_maps.append({
            "ab": np.ascontiguousarray(ab_c),
            "am": am_c,
        })

    nc = _build_device()
    res = run_bass_kernel_spmd(nc, in_maps, core_ids=list(range(NCORES)))
    _LAST_RESULT = res

    full = np.empty((B, Q, Q), np.float32)
    for c in range(NCORES):
        b, ih = c // 2, c % 2
        full[b, ih * RHALF : (ih + 1) * RHALF, :] = res.results[c]["out"]

    # ---- exact corrections for near-origin pairs (r < R0) ----
    ar = np.arange(Q)
    for b in range(B):
        u = x[b][:, None] - x[b][None, :]
        v = y[b][:, None] - y[b][None, :]
        mm = am[b][:, None] & am[b][None, :]
        near = (u * u + v * v < FIT_R0 * FIT_R0) & mm
        near[ar, ar] = False
        ii, jj = np.nonzero(near)
        if ii.size:
            uu, vv = u[ii, jj], v[ii, jj]
            corr = _g_exact(uu, vv, w1, b1, w2, b2, w3, b3) \
                 - _series_eval(uu, vv, dc, mus, mvs, cs)
            full[b, ii, jj] += corr.astype(np.float32)
        # diagonal: 0 where active, -1e9 where masked
        full[b, ar, ar] = np.where(am[b], np.float32(0.0), np.float32(-1e9))

    return full


# revision 27
# speedup vs baseline: 1.2334x; 1.0585x over previous
"""Trainium2 Bass kernel for nn_DenseGeometryEdgeHead.

Math background
---------------
reference(q, coords, ...) computes, per batch b:

    logits[i,j] = (q W) q^T  +  g(dx_ij, dy_ij)  + bias,   then diag-zero + mask

where g() is a tiny MLP (6->64->64->1) applied to pairwise geometry features
that depend ONLY on (dx, dy) = p_i - p_j.

Because g is a function of a coordinate *difference*, it factors through
Fourier features:  g(u,v) ~= sum_m 2*Re[c_m e^{i(w_m . p_i)} e^{-i(w_m . p_j)}].
Each mode contributes a rank-2 term cos/sin(theta_i)*cos/sin(theta_j), so the
whole geo-MLP collapses into a low-rank bilinear form  Phi(p_i) . Psi(p_j).
Stacked with the q W q^T term this makes the ENTIRE kernel one matmul with
contraction K = 256 (bilinear) + 1 (constants) + 2*NMODE (fourier) <= 768.

The Fourier series is fit on the host per call (it depends only on the tiny
MLP weights): sample g on a grid, mollify the angular near-origin structure
below radius R0, taper for periodicity, FFT, keep the strongest NMODE modes.
Pairs with r < R0 (a ~2% sliver) get exact host-computed corrections that are
scatter-added into the output, so the approximation error lives only in the
smooth far-field where the truncated series is accurate.

Device work per core (data-parallel over (batch, row-half)):
    out[512,1024] = AT^T @ BT   (K=768, fp32 PSUM)  + active-mask -1e9 select
which is memory-bound (~7 MB of HBM traffic per core).
"""

import numpy as np

import concourse.bass as bass
import concourse.bacc as bacc
import concourse.mybir as mybir
import concourse.tile as tile
from concourse.bass_utils import run_bass_kernel_spmd

# ---- problem constants (hardcoded; harness runs kernel.py standalone) ----
B, Q, D, H = 4, 1024, 256, 64
NCORES = 8
NSLICE = 4               # K slices of 128  -> total contraction 512
KTOT = NSLICE * 128      # 512 = 256 (qW/q) + 1 (const) + 254 (127 modes) + 1 pad
NMODE = 127
RHALF = Q // 2           # output rows per core

# ---- fourier fit hyperparameters (host-side, tunable without recompiling) ----
FIT_NG = 512             # fit grid resolution
FIT_L = 1.28             # series half-period (u,v in [-1,1])
FIT_KMAX = 24            # candidate mode box
FIT_R0 = 0.22            # below this radius: mollified + exact corrections
FIT_RIN_FRAC = 0.5       # blend starts at R0*frac
FIT_TAPER_LO = 1.02      # periodicity taper window
FIT_TAPER_HI = 1.26
FIT_NSUB = 60000         # least-squares refit sample count
FIT_IRLS = 3             # IRLS iterations toward minimax

F32 = mybir.dt.float32
F32R = mybir.dt.float32r  # full-rate PE streaming (1 cyc/row vs 4 for fp32)

# True: q/qW contraction slices run in native fp32 (2-pass, slower, exact);
# False: everything float32r (~12-bit multiply mantissa, max abs err ~0.016)
PRECISE_BILINEAR = False

_LAST_RESULT = None      # BassKernelResults of the most recent device run
_LAST_FIT = None         # diagnostics from the most recent fourier fit
_NC_CACHE = None


# ======================= device kernel =======================

def _build_device():
    """One core's program: out[512,1024] = AT^T @ BT with mask/-1e9 select."""
    global _NC_CACHE
    if _NC_CACHE is not None:
        return _NC_CACHE
    nc = bacc.Bacc()  # Bacc.finalize() legalizes multi-wait instructions
    # combined per-K-slice chunks: [ks][p, 0:512]=AT slice, [p, 512:1536]=BT
    # slice — so each matmul depends on exactly ONE DMA (PE instructions
    # only support a single sync-wait).
    ab_d = nc.declare_dram_parameter("ab", [NSLICE, 128, RHALF + Q], F32R, False)
    # mask pack: cols [0:Q]=amj bcast, [Q]=-1e9, [Q+1:Q+5]=ami, [Q+5:Q+9]=amiadd
    am_d = nc.declare_dram_parameter("am", [128, Q + 9], F32, False)
    out_d = nc.declare_dram_parameter("out", [RHALF, Q], F32, True)

    Id = mybir.ActivationFunctionType.Identity
    mult = mybir.AluOpType.mult
    add = mybir.AluOpType.add

    with tile.TileContext(nc) as tc:
        with (
            tc.tile_pool(name="inp", bufs=1) as inp,
            tc.tile_pool(name="psum", bufs=3, space=bass.MemorySpace.PSUM) as psum,
            tc.tile_pool(name="work", bufs=3) as work,
        ):
            ats, bts = [], []
            for ks in range(NSLICE):
                dt_k = F32 if (PRECISE_BILINEAR and ks < 2) else F32R
                ab = inp.tile([128, RHALF + Q], dt_k, tag=f"ab{ks}")
                # split each chunk across BOTH HWDGE queues (sync + scalar)
                # so the load stream isn't serialized on one ring
                half = (RHALF + Q) // 2
                nc.sync.dma_start(ab[:, :half], ab_d[ks, :, :half])
                nc.scalar.dma_start(ab[:, half:], ab_d[ks, :, half:])
                ats.append(ab[:, :RHALF])
                bts.append(ab[:, RHALF:])
            am = inp.tile([128, Q + 9], F32, tag="am")
            nc.scalar.dma_start(am[:], am_d[:, :])
            colcap = am[:, :Q]  # +3e38 where col active, -1e9 where masked

            # pre-observe the mask DMA on the vector engine so later DVE ops
            # only carry the ACT-sem wait (1-wait limit per instruction)
            warm = work.tile([128, 1], F32, tag="warm")
            nc.vector.tensor_copy(warm[:], am[:, Q : Q + 1])

            import bass_rust as _br
            prev_last_mm = None
            for ib in range(4):
                isl = slice(ib * 128, (ib + 1) * 128)
                ami_c = am[:, Q + 1 + ib : Q + 2 + ib]
                amia_c = am[:, Q + 5 + ib : Q + 6 + ib]
                # both j-halves accumulate into one 2-bank psum tile
                ps = psum.tile([128, Q], F32, tag="ps")
                first_mm = last_mm = None
                for ks in range(NSLICE):
                    for jc in range(2):
                        jsl = slice(jc * RHALF, (jc + 1) * RHALF)
                        mm = nc.tensor.matmul(
                            ps[:, jsl],
                            ats[ks][:, isl],
                            bts[ks][:, jsl],
                            start=(ks == 0),
                            stop=(ks == NSLICE - 1),
                        )
                        last_mm = mm
                        if first_mm is None:
                            first_mm = mm
                # keep PE group-contiguous: without this the scheduler
                # round-robins the psum groups and no group's mask chain can
                # start until nearly the end
                if prev_last_mm is not None:
                    _br.add_dep_helper(
                        first_mm.ins, prev_last_mm.ins, True,
                        "serialize psum groups for early drain",
                    )
                prev_last_mm = last_mm
                # drain each j-half as its psum bank completes:
                # row mask on ACT straight out of PSUM:
                #   o1 = ps*ami + (ami-1)*1e9   ({logits or -1e9 rows})
                # col mask on DVE in ONE exact op:
                #   o = min(o1, colcap)  (colcap = +3e38 active / -1e9 masked)
                for jc in range(2):
                    jsl = slice(jc * RHALF, (jc + 1) * RHALF)
                    o1 = work.tile([128, RHALF], F32, tag="o1")
                    nc.scalar.activation(o1[:], ps[:, jsl], Id,
                                         scale=ami_c, bias=amia_c)
                    o = work.tile([128, RHALF], F32, tag="o")
                    nc.vector.tensor_tensor(o[:], o1[:], colcap[:, jsl],
                                            op=mybir.AluOpType.min)
                    nc.sync.dma_start(out_d[isl, jsl], o[:])
    nc.finalize()
    _NC_CACHE = nc
    return nc


# ======================= host-side math =======================

def _silu(x):
    return x * (1.0 / (1.0 + np.exp(-x)))


def _mlp(feats, w1, b1, w2, b2, w3, b3):
    """feats [..., 6] -> geo logits [...] (includes +b3)."""
    h = _silu(feats @ w1 + b1)
    h = _silu(h @ w2 + b2)
    return (h @ w3)[..., 0] + b3[0]


def _g_exact(u, v, w1, b1, w2, b2, w3, b3):
    """Exact reference-semantics geo MLP output for difference vectors."""
    r2 = u * u + v * v
    rd = np.sqrt(r2 + 1e-8)
    mz = (np.abs(u) < 1e-6) & (np.abs(v) < 1e-6)
    us = np.where(mz, 1e-6, u)
    vs = np.where(mz, 1e-6, v)
    ang = np.arctan2(vs, us)
    feats = np.stack([u, v, rd, r2, np.sin(ang), np.cos(ang)], axis=-1)
    return _mlp(feats, w1, b1, w2, b2, w3, b3)


def _qstep(t):
    t = np.clip(t, 0.0, 1.0)
    return t * t * t * (t * (6.0 * t - 15.0) + 10.0)


def _fit_fourier(w1, b1, w2, b2, w3, b3):
    """Fit truncated 2-D fourier series to g on [-L,L)^2.

    Returns (dc, mus, mvs, coeffs, diag) — dc real, modes in the half-plane
    with complex coeffs (term = 2*Re[c * e^{i pi (mu u + mv v)/L}]).
    """
    NG, L = FIT_NG, FIT_L
    du = 2.0 * L / NG
    ax = (np.arange(NG) - NG // 2) * du
    U = np.broadcast_to(ax[None, :], (NG, NG))      # u along columns
    V = np.broadcast_to(ax[:, None], (NG, NG))      # v along rows
    r = np.hypot(U, V)
    r2 = U * U + V * V
    rd = np.sqrt(r2 + 1e-8)

    # mollified angular features near origin
    blend = _qstep((r - FIT_R0 * FIT_RIN_FRAC) / (FIT_R0 * (1.0 - FIT_RIN_FRAC)))
    rsafe = np.maximum(r, 1e-30)
    sb = np.float64(np.sqrt(2.0) / 2.0)
    s_m = blend * (V / rsafe) + (1.0 - blend) * sb
    c_m = blend * (U / rsafe) + (1.0 - blend) * sb

    feats = np.stack([U, V, rd, r2, s_m, c_m], axis=-1).astype(np.float32)
    g_m = _mlp(feats.reshape(-1, 6), w1.astype(np.float32), b1.astype(np.float32),
               w2.astype(np.float32), b2.astype(np.float32),
               w3.astype(np.float32), b3.astype(np.float32)).reshape(NG, NG)
    g_m = g_m.astype(np.float64)

    # periodicity taper
    wu = 1.0 - _qstep((np.abs(ax) - FIT_TAPER_LO) / (FIT_TAPER_HI - FIT_TAPER_LO))
    Wt = wu[:, None] * wu[None, :]
    ring = (Wt > 0.05) & (Wt < 0.95)
    C = float(np.mean(g_m[ring])) if ring.any() else float(np.mean(g_m))
    gp = Wt * (g_m - C) + C

    F = np.fft.fft2(gp) / (NG * NG)
    fr = np.fft.fftfreq(NG, d=1.0 / NG).astype(np.int64)  # signed integer freqs
    # c_{mv,mu} = F[m,n] * (-1)^(m~+n~);  series = sum c e^{i pi (mu u + mv v)/L}
    sgn = np.where(fr % 2 == 0, 1.0, -1.0)
    Csym = F * sgn[:, None] * sgn[None, :]

    # candidate half-plane modes within the KMAX box
    KM = FIT_KMAX
    cand = []
    for mi, mv in enumerate(fr):
        if not (0 <= mv <= KM):
            continue
        for ni, mu in enumerate(fr):
            if abs(mu) > KM:
                continue
            if mv == 0 and mu <= 0:
                continue
            cand.append((abs(Csym[mi, ni]), mu, mv, Csym[mi, ni]))
    cand.sort(key=lambda t: -t[0])
    sel = cand[:NMODE]
    mus = np.array([t[1] for t in sel], np.int64)
    mvs = np.array([t[2] for t in sel], np.int64)
    dc = float(Csym[0, 0].real)

    # ---- least-squares refit of (dc, c_m) on the region that matters:
    # {r >= R0, |u|,|v| <= 1}; FFT coeffs optimize the wrong (periodic) norm.
    far = (r >= FIT_R0) & (np.abs(U) <= 1.0) & (np.abs(V) <= 1.0)
    g_true = _g_exact(U[far], V[far], w1, b1, w2, b2, w3, b3)
    rng = np.random.default_rng(12345)
    nfar = g_true.size
    sub = rng.choice(nfar, size=min(FIT_NSUB, nfar), replace=False)
    uu, vv, gg = U[far][sub], V[far][sub], g_true[sub]
    w = np.pi / L
    PH = np.exp(1j * w * (np.outer(uu, mus) + np.outer(vv, mvs)))
    # 2*Re[c e^{i th}] = 2*Re(c)*cos - 2*Im(c)*sin
    X = np.concatenate([np.ones((uu.size, 1)), 2 * PH.real, -2 * PH.imag], axis=1)
    wt = np.ones(uu.size)
    sol = None
    for _ in range(1 + FIT_IRLS):
        Xw = X * wt[:, None]
        G = Xw.T @ X
        sol = np.linalg.solve(G + 1e-9 * np.eye(G.shape[0]), Xw.T @ gg)
        err = X @ sol - gg
        wt = (np.abs(err) + 1e-4)
    dc = float(sol[0])
    cs = sol[1 : 1 + NMODE] + 1j * sol[1 + NMODE :]
    err = X @ sol - gg
    diag = dict(max_err=float(np.abs(err).max()),
                p99_err=float(np.quantile(np.abs(err), 0.99)))
    return dc, mus, mvs, cs, diag


def _series_eval(u, v, dc, mus, mvs, cs):
    """Evaluate the selected truncated series at difference vectors (f64)."""
    out = np.full(u.shape, dc, np.float64)
    w = np.pi / FIT_L
    step = 1 << 14
    for s in range(0, u.size, step):
        sl = slice(s, min(s + step, u.size))
        ph = np.exp(1j * w * (np.outer(u.ravel()[sl], mus) + np.outer(v.ravel()[sl], mvs)))
        out.ravel()[sl] += 2.0 * np.real(ph @ cs)
    return out


# ======================= kernel entry =======================

def kernel(q, coords, active_mask, W, w1, b1, w2, b2, w3, b3, bias):
    global _LAST_RESULT, _LAST_FIT
    q = np.asarray(q, np.float32)
    coords = np.asarray(coords, np.float32)
    am = np.asarray(active_mask).astype(bool)
    W = np.asarray(W, np.float64)
    w1, b1 = np.asarray(w1, np.float64), np.asarray(b1, np.float64)
    w2, b2 = np.asarray(w2, np.float64), np.asarray(b2, np.float64)
    w3, b3 = np.asarray(w3, np.float64), np.asarray(b3, np.float64)
    bias0 = float(np.asarray(bias).reshape(-1)[0])

    # ---- fourier factorization of the geo MLP ----
    dc, mus, mvs, cs, fit_diag = _fit_fourier(w1, b1, w2, b2, w3, b3)
    _LAST_FIT = fit_diag

    # ---- per-point feature maps ----
    # AT rows: [qW^T (256) | const row | 2rho cos(th+g), 2rho sin(th+g) | pad]
    # BT rows: [q^T  (256) | ones      | cos(th), sin(th)              | pad]
    qW = (q.astype(np.float64) @ W).astype(np.float64)          # [B,Q,D]
    x = coords[..., 0].astype(np.float64)                        # [B,Q]
    y = coords[..., 1].astype(np.float64)
    wfreq = np.pi / FIT_L
    theta = wfreq * (x[..., None] * mus + y[..., None] * mvs)    # [B,Q,NMODE]
    rho = np.abs(cs)
    gam = np.angle(cs)
    AT = np.zeros((B, KTOT, Q), np.float32)
    BT = np.zeros((B, KTOT, Q), np.float32)
    for b in range(B):
        AT[b, :D, :] = qW[b].T.astype(np.float32)
        BT[b, :D, :] = q[b].T
        AT[b, D, :] = np.float32(dc + b3[0] + bias0)
        BT[b, D, :] = 1.0
        AT[b, D + 1 : D + 1 + NMODE, :] = (2.0 * rho[:, None] * np.cos(theta[b].T + gam[:, None])).astype(np.float32)
        AT[b, D + 1 + NMODE : D + 1 + 2 * NMODE, :] = (2.0 * rho[:, None] * np.sin(theta[b].T + gam[:, None])).astype(np.float32)
        BT[b, D + 1 : D + 1 + NMODE, :] = np.cos(theta[b].T).astype(np.float32)
        BT[b, D + 1 + NMODE : D + 1 + 2 * NMODE, :] = np.sin(theta[b].T).astype(np.float32)

    # ---- per-core inputs: core c -> (batch c//2, row half c%2) ----
    amf = am.astype(np.float32)
    in_maps = []
    for c in range(NCORES):
        b, ih = c // 2, c % 2
        rs = slice(ih * RHALF, (ih + 1) * RHALF)
        ami = amf[b, rs].reshape(4, 128).T                       # [128,4]
        ab_c = np.concatenate(
            [AT[b, :, rs].reshape(NSLICE, 128, RHALF), BT[b].reshape(NSLICE, 128, Q)],
            axis=2,
        )
        am_c = np.empty((128, Q + 9), np.float32)
        am_c[:, :Q] = np.where(amf[b] > 0.5, np.float32(3e38), np.float32(-1e9))
        am_c[:, Q] = -1e9
        am_c[:, Q + 1 : Q + 5] = ami
        am_c[:, Q + 5 : Q + 9] = (ami - 1.0) * 1e9
        in_maps.append({
            "ab": np.ascontiguousarray(ab_c),
            "am": am_c,
        })

    nc = _build_device()
    res = run_bass_kernel_spmd(nc, in_maps, core_ids=list(range(NCORES)))
    _LAST_RESULT = res

    full = np.empty((B, Q, Q), np.float32)
    for c in range(NCORES):
        b, ih = c // 2, c % 2
        full[b, ih * RHALF : (ih + 1) * RHALF, :] = res.results[c]["out"]

    # ---- exact corrections for near-origin pairs (r < R0) ----
    ar = np.arange(Q)
    for b in range(B):
        u = x[b][:, None] - x[b][None, :]
        v = y[b][:, None] - y[b][None, :]
        mm = am[b][:, None] & am[b][None, :]
        near = (u * u + v * v < FIT_R0 * FIT_R0) & mm
        near[ar, ar] = False
        ii, jj = np.nonzero(near)
        if ii.size:
            uu, vv = u[ii, jj], v[ii, jj]
            corr = _g_exact(uu, vv, w1, b1, w2, b2, w3, b3) \
                 - _series_eval(uu, vv, dc, mus, mvs, cs)
            full[b, ii, jj] += corr.astype(np.float32)
        # diagonal: 0 where active, -1e9 where masked
        full[b, ar, ar] = np.where(am[b], np.float32(0.0), np.float32(-1e9))

    return full
